# revision 8
# baseline (speedup 1.0000x reference)
"""Trainium2 Bass kernel for a dense attention layer.

Reference computation (B=4, Q=K=4096, IN=D=256):
    q = queries @ Wq.T + bq ; k = keys @ Wk.T + bk ; v = values @ Wv.T + bv
    scores = (q @ k.T  masked to key < mask[q] with -1e9) / sqrt(D)
    out = softmax(scores) @ v @ Wo.T + bo

Strategy (v2):
  - Data-parallel: core c handles batch b = c//2, half of the queries,
    sorted by mask length and dealt round-robin (as v1).
  - Wo is folded into the V projection on the host (W2 = Wo @ Wv,
    b' = Wo @ bv + bo): out = (P @ V') / den + b', eliminating the output
    projection, PE transposes and two PSUM->SBUF copies per subtile.
  - fp8e4 DoubleRow matmuls everywhere:
      * K/Q projections contract the 256 input dims in one DR matmul
        (raw activations and Wq/Wk shipped as fp8).
      * scores as in v1 (fp8 qT/kT, 256-dim DR contraction).
      * PV: probs in fp8 (written directly by the exp), V' split as
        A + B with A = fp8(V'), B = fp8(V' - A) -- two DR matmuls over
        chunk PAIRS (256-key contraction) cost half of one bf16 PV while
        keeping full bf16-level precision of V'.
  - Biases ride free on the mandatory PSUM->SBUF projection copies
    (per-partition scalar operands are free in tensor_scalar).
  - exp is issued once per chunk pair; mask validity tiles are
    precomputed on the host and shipped as one small fp8 tensor.
  - Epilogue per 128-query subtile: one reciprocal (per tile) + one
    fused (att * rec + b') scalar_tensor_tensor, then DMA.
  Measured numerics (numpy emulation): rel err ~1.1e-2 (gate 2e-2).
"""

import bisect
import numpy as np

import concourse.bass as bass
import concourse.mybir as mybir
from concourse import bacc
from concourse.tile import TileContext
from concourse.bass_utils import run_bass_kernel_spmd

F32 = mybir.dt.float32
F8 = mybir.dt.float8e4
BF = mybir.dt.bfloat16

BF16NP = mybir.dt.np(BF)
F8NP = mybir.dt.np(F8)

B, Q, KLEN, IN, D = 4, 4096, 4096, 256, 256
N_CORES = 8
QS = Q // 2            # queries per core
TQ = 512               # query tile
NQT = QS // TQ         # query tiles per core
KC = 128               # key chunk
NKC = KLEN // KC
NS = TQ // 128         # query subtiles per tile
SCALE = 1.0 / 16.0     # 1/sqrt(D)


def _make_plan(sorted_masks):
    """sorted_masks: [N_CORES, QS] ascending per-core mask lengths.

    Returns (n_chunks, zs, es, vd_off, vd_total):
      n_chunks[t]: key chunks needed for query tile t (max over cores)
      zs[t][j] = (zq, zx): aligned / exact first active query column
      es[t][j]: end of the mask-multiply column range (e == zx -> none)
      vd_off[(t, j)]: column offset of the validity block in the vd blob
    """
    n_chunks, zs, es = [], [], []
    vd_off, off = {}, 0
    for t in range(NQT):
        seg = sorted_masks[:, t * TQ:(t + 1) * TQ]
        nc_t = int(np.ceil(seg.max() / KC))
        ztj, etj = [], []
        for j in range(nc_t):
            z = int(min(np.searchsorted(seg[c], KC * j, side="right")
                        for c in range(N_CORES)))
            e = int(max(np.searchsorted(seg[c], KC * (j + 1), side="left")
                        for c in range(N_CORES)))
            e = max(e, z)
            zq = (z // 128) * 128
            ztj.append((zq, z))
            etj.append(e)
            if e > z:
                vd_off[(t, j)] = off
                off += e - z
        n_chunks.append(nc_t)
        zs.append(ztj)
        es.append(etj)
    return n_chunks, zs, es, vd_off, max(off, 1)


def _bcast_ap(handle, parts, free):
    ap = handle.ap()
    return bass.AP(tensor=ap.tensor, offset=ap.offset, ap=[[0, parts], [1, free]])


def build_bass(plan, pipe=3):
    n_chunks, zs, es, vd_off, vd_total = plan
    nc = bacc.Bacc(
        "TRN2",
        target_bir_lowering=False,
        debug=False,
        enable_asserts=False,
        num_devices=1,
    )

    qT_d = nc.declare_dram_parameter("qT", [2, 128, QS], F8, isOutput=False)
    kT_d = nc.declare_dram_parameter("kT", [2, 128, KLEN], F8, isOutput=False)
    vT_d = nc.declare_dram_parameter("vT", [2, 128, KLEN], BF, isOutput=False)
    WqT_d = nc.declare_dram_parameter("WqT", [2, 128, D], F8, isOutput=False)
    WkT_d = nc.declare_dram_parameter("WkT", [2, 128, D], F8, isOutput=False)
    W2T_d = nc.declare_dram_parameter("W2T", [2, 128, D], BF, isOutput=False)
    bq_d = nc.declare_dram_parameter("bqS", [2, 128], F32, isOutput=False)
    bk_d = nc.declare_dram_parameter("bk", [2, 128], F32, isOutput=False)
    bp_d = nc.declare_dram_parameter("bp", [1, D], F32, isOutput=False)
    vd_d = nc.declare_dram_parameter("vd", [128, vd_total], F8, isOutput=False)
    out_d = nc.declare_dram_parameter("out", [QS, D], BF, isOutput=True)

    with TileContext(nc) as tc:
        with (
            tc.tile_pool(name="consts", bufs=1) as consts,
            tc.tile_pool(name="probs", bufs=5) as probs,
            tc.tile_pool(name="recp", bufs=2) as recp,
            tc.tile_pool(name="outsb", bufs=2) as outsb,
            tc.tile_pool(name="scps", bufs=2, space="PSUM") as scps,
            tc.tile_pool(name="attps", bufs=1, space="PSUM") as attps,
        ):
            # ---- constants ------------------------------------------------
            WkT_s = consts.tile([128, 2, D], F8, tag="WkT")
            WqT_s = consts.tile([128, 2, D], F8, tag="WqT")
            W2T_s = consts.tile([128, 2, D], BF, tag="W2T")
            bq_s = consts.tile([128, 2], F32, tag="bq")
            bk_s = consts.tile([128, 2], F32, tag="bk")
            bp_s = consts.tile([128, D], F32, tag="bp")
            vd_s = consts.tile([128, vd_total], F8, tag="vd")

            # raw activations, grouped so compute can start early
            KB = [0, 1024, 2560, 4096]
            QB = [0, 1024, 2048]
            VB = [0, 1024, 2048, 3072, 4096]

            def raw_tiles(prefix, bounds, dt):
                return [consts.tile([128, 2, bounds[i + 1] - bounds[i]], dt,
                                    tag=f"{prefix}{i}", name=f"{prefix}{i}")
                        for i in range(len(bounds) - 1)]

            kraw = raw_tiles("kraw", KB, F8)
            qraw = raw_tiles("qraw", QB, F8)
            vraw = raw_tiles("vraw", VB, BF)

            def raw_slice(tiles, bounds, lo, hi, c=None):
                g = bisect.bisect_right(bounds, lo) - 1
                assert hi <= bounds[g + 1], (lo, hi, bounds)
                if c is None:
                    return tiles[g][:, :, lo - bounds[g]:hi - bounds[g]]
                return tiles[g][:, c, lo - bounds[g]:hi - bounds[g]]

            def raw_dma(eng, tiles, bounds, dram, g):
                eng.dma_start(out=tiles[g][:, :, :],
                              in_=dram[:, :, bounds[g]:bounds[g + 1]].rearrange(
                                  "c p q -> p c q"))

            # DMA issue order per queue = arrival priority
            # (queues: sync=HWDGE, scalar=Act SEQ early only, gpsimd=SWDGE)
            nc.sync.dma_start(out=WkT_s[:, :, :],
                              in_=WkT_d.rearrange("c p d -> p c d"))
            nc.scalar.dma_start(out=bk_s[:, :], in_=bk_d.rearrange("c p -> p c"))
            nc.gpsimd.dma_start(out=vd_s[:, :], in_=vd_d[:, :])
            raw_dma(nc.sync, kraw, KB, kT_d, 0)
            nc.scalar.dma_start(out=bq_s[:, :], in_=bq_d.rearrange("c p -> p c"))
            nc.scalar.dma_start(out=WqT_s[:, :, :],
                                in_=WqT_d.rearrange("c p d -> p c d"))
            raw_dma(nc.gpsimd, qraw, QB, qT_d, 0)
            nc.sync.dma_start(out=W2T_s[:, :, :],
                              in_=W2T_d.rearrange("c p d -> p c d"))
            nc.scalar.dma_start(out=bp_s[:, :], in_=_bcast_ap(bp_d, 128, D))
            raw_dma(nc.gpsimd, qraw, QB, qT_d, 1)
            raw_dma(nc.sync, vraw, VB, vT_d, 0)
            raw_dma(nc.sync, kraw, KB, kT_d, 1)
            raw_dma(nc.scalar, vraw, VB, vT_d, 1)
            raw_dma(nc.sync, kraw, KB, kT_d, 2)
            raw_dma(nc.gpsimd, vraw, VB, vT_d, 2)
            raw_dma(nc.gpsimd, vraw, VB, vT_d, 3)

            # ---- projections ----------------------------------------------
            # K first (scores need kT earliest); copies carry the bias.
            kT_s = consts.tile([128, 2, KLEN], F8, tag="kT")
            qT_s = consts.tile([128, 2, QS], F8, tag="qT")
            vA = consts.tile([128, NKC, D + 1], F8, tag="vA")
            vB = consts.tile([128, NKC, D + 1], F8, tag="vB")
            nc.gpsimd.memset(vA[:, :, D:D + 1], 1.0)
            nc.vector.memset(vB[:, :, D:D + 1], 0.0)

            # PSUM->SBUF traffic is DVE/Act only (GPSIMD cannot touch PSUM).
            # Act carries the exp wall, so K copies go to DVE, Q to Act.
            def k_proj(kt):
                for dd in range(2):
                    ps = scps.tile([128, TQ], F32, tag="sc")
                    nc.tensor.matmul(ps[:, :],
                                     WkT_s[:, :, dd * 128:(dd + 1) * 128],
                                     raw_slice(kraw, KB, kt * 512, (kt + 1) * 512),
                                     start=True, stop=True,
                                     perf_mode=mybir.MatmulPerfMode.DoubleRow)
                    nc.vector.tensor_scalar(
                        kT_s[:, dd, kt * 512:(kt + 1) * 512], ps[:, :],
                        bk_s[:, dd:dd + 1], None, mybir.AluOpType.add)

            def q_proj(kt):
                for dd in range(2):
                    ps = scps.tile([128, TQ], F32, tag="sc")
                    nc.tensor.matmul(ps[:, :],
                                     WqT_s[:, :, dd * 128:(dd + 1) * 128],
                                     raw_slice(qraw, QB, kt * 512, (kt + 1) * 512),
                                     start=True, stop=True,
                                     perf_mode=mybir.MatmulPerfMode.DoubleRow)
                    # qT = (psum + bq) * SCALE = psum * SCALE + bqS
                    nc.scalar.activation(
                        qT_s[:, dd, kt * 512:(kt + 1) * 512], ps[:, :],
                        mybir.ActivationFunctionType.Identity,
                        bias=bq_s[:, dd:dd + 1], scale=SCALE)

            def v_proj(g):
                # four consecutive key chunks share one 2-bank PSUM tile so
                # the A copy / B residual run as single 1024-col DVE ops
                ps4 = scps.tile([128, 4, D], F32, tag="sc")
                for jj in range(4):
                    j = 4 * g + jj
                    for c in range(2):
                        nc.tensor.matmul(ps4[:, jj, :],
                                         raw_slice(vraw, VB, j * 128,
                                                   (j + 1) * 128, c),
                                         W2T_s[:, c, :],
                                         start=(c == 0), stop=(c == 1))
                j0 = 4 * g
                nc.vector.tensor_copy(out=vA[:, j0:j0 + 4, 0:D],
                                      in_=ps4[:, :, :])
                nc.vector.scalar_tensor_tensor(
                    out=vB[:, j0:j0 + 4, 0:D], in0=ps4[:, :, :], scalar=0.0,
                    in1=vA[:, j0:j0 + 4, 0:D],
                    op0=mybir.AluOpType.bypass, op1=mybir.AluOpType.subtract)

            for kt in range(KLEN // 512):
                k_proj(kt)
            for kt in range(QS // 512):
                q_proj(kt)
            for g in range(NKC // 4):
                v_proj(g)

            # ---- attention ------------------------------------------------
            ep_queue = []

            def make_epilogue(t, att4):
                q0 = t * TQ
                rec4 = recp.tile([128, NS], F32, tag="rec")
                ot = outsb.tile([128, NS, D], BF, tag="ot")
                ops = [lambda: nc.vector.reciprocal(rec4[:, :], att4[:, :, D])]
                for s in range(NS):
                    def c1(s=s):
                        nc.vector.scalar_tensor_tensor(
                            out=ot[:, s, :], in0=att4[:, s, 0:D],
                            scalar=rec4[:, s:s + 1], in1=bp_s[:, :],
                            op0=mybir.AluOpType.mult, op1=mybir.AluOpType.add)
                    ops.append(c1)

                def c2():
                    out_slice = out_d[q0:q0 + TQ, :].rearrange(
                        "(s p) d -> p s d", p=128)
                    nc.sync.dma_start(out=out_slice, in_=ot[:, :, :])
                ops.append(c2)
                return ops

            for t in range(NQT):
                nch = n_chunks[t]
                q0 = t * TQ
                att4 = attps.tile([128, NS, TQ], F32, tag="att4")

                # chunk pairs for DoubleRow PV
                pairs = []
                for m in range((nch + 1) // 2):
                    j0, j1 = 2 * m, 2 * m + 1
                    if j1 >= nch:
                        j1 = None
                    zp = zs[t][j0][0]
                    pairs.append((j0, j1, zp))
                last_m = [max(m for m, (_, _, zp) in enumerate(pairs)
                              if zp < (s + 1) * 128) for s in range(NS)]

                pending = []

                def issue_pv(m, pb8, zp, att4=att4, last_m=last_m):
                    for s in range(zp // 128, NS):
                        nc.tensor.matmul(att4[:, s, 0:D + 1],
                                         pb8[:, :, s * 128:(s + 1) * 128],
                                         vA[:, 2 * m:2 * m + 2, :],
                                         start=(m == 0), stop=False,
                                         perf_mode=mybir.MatmulPerfMode.DoubleRow)
                        nc.tensor.matmul(att4[:, s, 0:D + 1],
                                         pb8[:, :, s * 128:(s + 1) * 128],
                                         vB[:, 2 * m:2 * m + 2, :],
                                         start=False, stop=(m == last_m[s]),
                                         perf_mode=mybir.MatmulPerfMode.DoubleRow)

                for m, (j0, j1, zp) in enumerate(pairs):
                    (zq0, zx0), e0 = zs[t][j0], es[t][j0]
                    sc2 = scps.tile([128, 2, TQ], F32, tag="sc")
                    nc.tensor.matmul(sc2[:, 0, zx0:],
                                     kT_s[:, :, j0 * 128:(j0 + 1) * 128],
                                     qT_s[:, :, q0 + zx0:q0 + TQ],
                                     start=True, stop=True,
                                     perf_mode=mybir.MatmulPerfMode.DoubleRow)
                    if j1 is not None:
                        nc.tensor.matmul(sc2[:, 1, zx0:],
                                         kT_s[:, :, j1 * 128:(j1 + 1) * 128],
                                         qT_s[:, :, q0 + zx0:q0 + TQ],
                                         start=True, stop=True,
                                         perf_mode=mybir.MatmulPerfMode.DoubleRow)
                    pb8 = probs.tile([128, 2, TQ], F8, tag="pb")
                    if j1 is not None:
                        nc.scalar.activation(pb8[:, :, zx0:], sc2[:, :, zx0:],
                                             mybir.ActivationFunctionType.Exp)
                    else:
                        nc.scalar.activation(pb8[:, 0, zx0:], sc2[:, 0, zx0:],
                                             mybir.ActivationFunctionType.Exp)
                        nc.gpsimd.memset(pb8[:, 1, zp:], 0.0)
                    if zx0 > zp:
                        nc.gpsimd.memset(pb8[:, 0, zp:zx0], 0.0)
                    if e0 > zx0:
                        off = vd_off[(t, j0)] + (zx0 - zs[t][j0][1])
                        nc.gpsimd.tensor_mul(pb8[:, 0, zx0:e0], pb8[:, 0, zx0:e0],
                                             vd_s[:, off:off + e0 - zx0])
                    if j1 is not None:
                        (zq1, zx1), e1 = zs[t][j1], es[t][j1]
                        if zx1 > zp:
                            nc.gpsimd.memset(pb8[:, 1, zp:zx1], 0.0)
                        if e1 > zx1:
                            off = vd_off[(t, j1)]
                            nc.gpsimd.tensor_mul(pb8[:, 1, zx1:e1],
                                                 pb8[:, 1, zx1:e1],
                                                 vd_s[:, off:off + e1 - zx1])
                    for _ in range(2):
                        if ep_queue:
                            ep_queue.pop(0)()
                    pending.append((m, pb8, zp))
                    if len(pending) > pipe:
                        issue_pv(*pending.pop(0))
                while pending:
                    issue_pv(*pending.pop(0))
                while ep_queue:
                    ep_queue.pop(0)()
                ep_queue = make_epilogue(t, att4)
            while ep_queue:
                ep_queue.pop(0)()

    nc.compile()
    return nc


def prepare(inputs):
    """Host-side sharding. Returns (in_maps, plan, perms)."""
    queries = np.asarray(inputs["queries"], np.float32)
    keys = np.asarray(inputs["keys"], np.float32)
    values = np.asarray(inputs["values"], np.float32)
    mask = np.asarray(inputs["mask"])
    w = {k: np.asarray(inputs[k], np.float32)
         for k in ("Wq", "bq", "Wk", "bk", "Wv", "bv", "Wo", "bo")}

    W2 = (w["Wo"].astype(np.float64) @ w["Wv"].astype(np.float64)).astype(np.float32)
    bp = (w["Wo"] @ w["bv"] + w["bo"]).astype(np.float32)
    shared = {
        "WqT": np.ascontiguousarray(w["Wq"].T).reshape(2, 128, D).astype(F8NP),
        "WkT": np.ascontiguousarray(w["Wk"].T).reshape(2, 128, D).astype(F8NP),
        "W2T": np.ascontiguousarray(W2.T).reshape(2, 128, D).astype(BF16NP),
        "bqS": (SCALE * w["bq"]).reshape(2, 128),
        "bk": w["bk"].reshape(2, 128),
        "bp": bp.reshape(1, D),
    }

    in_maps, perms = [], []
    sorted_masks = np.zeros((N_CORES, QS), np.int64)
    per_core = []
    for b in range(B):
        order = np.argsort(mask[b], kind="stable")
        keysT = np.ascontiguousarray(keys[b].T).reshape(2, 128, KLEN).astype(F8NP)
        valsT = np.ascontiguousarray(values[b].T).reshape(2, 128, KLEN).astype(BF16NP)
        for h in range(2):
            c = 2 * b + h
            idx = order[h::2]
            perms.append(idx)
            sorted_masks[c] = mask[b][idx]
            qT = np.ascontiguousarray(queries[b][idx].T)
            per_core.append({
                "qT": qT.reshape(2, 128, QS).astype(F8NP),
                "kT": keysT,
                "vT": valsT,
                **shared,
            })
    plan = _make_plan(sorted_masks)
    n_chunks, zs, es, vd_off, vd_total = plan

    key_ids = np.arange(128)
    for c in range(N_CORES):
        vd = np.zeros((128, vd_total), np.float32)
        for (t, j), off in vd_off.items():
            z, e = zs[t][j][1], es[t][j]
            mvals = sorted_masks[c, t * TQ + z:t * TQ + e]
            vd[:, off:off + e - z] = (mvals[None, :] > KC * j + key_ids[:, None])
        per_core[c]["vd"] = vd.astype(F8NP)
        in_maps.append(per_core[c])
    return in_maps, plan, perms


def assemble(results, perms):
    out = np.zeros((B, Q, D), np.float32)
    for c in range(N_CORES):
        out[c // 2][perms[c]] = np.asarray(results[c]["out"], np.float32)
    return out


def kernel(**inputs) -> np.ndarray:
    in_maps, plan, perms = prepare(inputs)
    nc = build_bass(plan)
    res = run_bass_kernel_spmd(nc, in_maps, core_ids=list(range(N_CORES)))
    return assemble(res.results, perms)
